# revision 1
# baseline (speedup 1.0000x reference)
"""Trainium2 Bass kernel for nn_O3TensorProductWeighted.

Computes, for each sample e:
    h  = relu(relu(weight @ W0 + b0) @ W1 + b1)           # [64]
    w  = h @ W2 + b2                                      # [36864] (never materialized)
    out0 = PW0*(einsum(Wa,s1)*s2 + I3*einsum(Wd,dot12))
    out1 = PW1*I3*(einsum(Wb,s1) x v2 + einsum(Wc,v1)*s2)
    out  = concat(out0, out1)/SQRT_K ; out[:128] += bias

Strategy: reassociate each einsum against the (k,u)-joint contraction of the
per-sample Khatri-Rao product h (x) x, so everything becomes dense matmuls
over shared W2 chunk weights, with the per-sample products built on-chip.
All paths share 32 paired h-row broadcasts (PE replicated-W1-column matmuls
+ ACT relu evacuation), one fused DVE multiply per chunk produces the six
path operands at once, and four PSUM accumulation chains collect the
outputs. Runs in transposed layout (features on partitions, samples on the
free dim), 512 samples per core, pure data parallel over 8 cores.
"""

import dataclasses
import sys

sys.path.insert(0, "/opt/trn_rl_repo")

from contextlib import ExitStack

import ml_dtypes
import numpy as np

import concourse.bacc as bacc
import concourse.bass as bass
import concourse.tile as tile
from concourse import mybir
from concourse.bass_utils import run_bass_kernel_spmd

BF16 = mybir.dt.bfloat16
F32 = mybir.dt.float32
BF16_NP = ml_dtypes.bfloat16

N_CORES = 8
N = 4096
E = N // N_CORES  # 512 samples per core

MUL0, MUL1 = 128, 64
N1 = MUL0 * MUL0          # 16384
N2 = MUL0 * MUL1          # 8192
N3 = MUL1 * MUL1          # 4096
FAN = MUL0 + MUL1         # 192
I3 = float(1.0 / np.sqrt(3.0))
# PW0/SQRT_K == 1.0 and PW1*I3/SQRT_K == 1.0 exactly; only I3 remains on D.

G = 32  # chunks; chunk g covers k in {2g, 2g+1} x 64 u-values (128 rows)


def _build_nc():
    nc = bacc.Bacc(None)

    # per-core inputs, transposed [feature, E]
    s1t_d = nc.declare_dram_parameter("s1t", [128, E], BF16, isOutput=False)
    # fused TT operand: [s1lo2 | s1hi2 | vs0 | vs1 | vs2 | d2], each [128, E]
    fin0_d = nc.declare_dram_parameter("fin0", [128, 6 * E], BF16, isOutput=False)
    wT_d = nc.declare_dram_parameter("wT", [16, E], BF16, isOutput=False)
    s2b_d = nc.declare_dram_parameter("s2b", [128, E], F32, isOutput=False)
    v2b_d = [
        nc.declare_dram_parameter(f"v2b{i}", [64, E], F32, isOutput=False)
        for i in range(3)
    ]

    # replicated parameters
    w0_d = nc.declare_dram_parameter("w0", [16, 64], BF16, isOutput=False)
    b0c_d = nc.declare_dram_parameter("b0c", [64, 1], F32, isOutput=False)
    wg1_d = nc.declare_dram_parameter("wg1", [64, G * 128], BF16, isOutput=False)
    bg1_d = nc.declare_dram_parameter("bg1", [128, G], F32, isOutput=False)
    walo_d = nc.declare_dram_parameter("walo", [128, G * 128], BF16, isOutput=False)
    wahi_d = nc.declare_dram_parameter("wahi", [128, G * 128], BF16, isOutput=False)
    wblo_d = nc.declare_dram_parameter("wblo", [128, G * 64], BF16, isOutput=False)
    wbhi_d = nc.declare_dram_parameter("wbhi", [128, G * 64], BF16, isOutput=False)
    wc_d = nc.declare_dram_parameter("wc", [128, G * 64], BF16, isOutput=False)
    wd_d = nc.declare_dram_parameter("wd", [128, G * 128], BF16, isOutput=False)
    ba_d = nc.declare_dram_parameter("ba", [128, 128], BF16, isOutput=False)
    bb_d = nc.declare_dram_parameter("bb", [128, 64], BF16, isOutput=False)
    bc_d = nc.declare_dram_parameter("bc", [64, 64], BF16, isOutput=False)
    bd_d = nc.declare_dram_parameter("bd", [64, 128], BF16, isOutput=False)
    bcol_d = nc.declare_dram_parameter("bcol", [128, 1], F32, isOutput=False)
    ident_d = nc.declare_dram_parameter("ident", [128, 128], F32, isOutput=False)

    outp_d = nc.declare_dram_parameter("outp", [E, 320], F32, isOutput=True)

    with tile.TileContext(nc) as tc, ExitStack() as ctx:
        const = ctx.enter_context(tc.tile_pool(name="const", bufs=1))
        work = ctx.enter_context(tc.tile_pool(name="work", bufs=1))
        bct_pool = ctx.enter_context(tc.tile_pool(name="bct", bufs=4))
        pt_pool = ctx.enter_context(tc.tile_pool(name="pt", bufs=4))
        out_pool = ctx.enter_context(tc.tile_pool(name="outs", bufs=2))
        ps_acc = ctx.enter_context(tc.tile_pool(name="ps_acc", bufs=1, space="PSUM"))
        ps_rot = ctx.enter_context(tc.tile_pool(name="ps_rot", bufs=2, space="PSUM"))

        dma_engines = [nc.sync, nc.gpsimd]
        dma_i = [0]

        def load(dparam, engine=None):
            t = const.tile(dparam.shape, dparam.dtype, name=f"t_{dparam.name}")
            e = engine
            if e is None:
                e = dma_engines[dma_i[0] % len(dma_engines)]
                dma_i[0] += 1
            e.dma_start(t[:], dparam[:])
            return t

        # small inputs first so the MLP + chunk 0 can start immediately
        wT_t = load(wT_d)
        w0_t = load(w0_d, nc.sync)
        b0c_t = load(b0c_d, nc.sync)
        wg1_t = load(wg1_d, nc.sync)
        bg1_t = load(bg1_d, nc.sync)
        fin0_t = load(fin0_d)
        s1t_t = load(s1t_d)
        ba_t = load(ba_d)
        bb_t = load(bb_d)
        bc_t = load(bc_d)
        bd_t = load(bd_d)
        walo_t = load(walo_d, nc.gpsimd)
        wblo_t = load(wblo_d, nc.sync)
        wahi_t = load(wahi_d, nc.gpsimd)
        wbhi_t = load(wbhi_d, nc.sync)
        wc_t = load(wc_d, nc.sync)
        wd_t = load(wd_d, nc.gpsimd)
        s2b_t = load(s2b_d)
        v2b_t = [load(d) for d in v2b_d]
        bcol_t = load(bcol_d)
        ident_t = load(ident_d)

        # MLP layer 1: h1 = relu(W0.T @ wT + b0) : [64, E]
        ps_h1 = ps_rot.tile([64, E], F32, tag="rot")
        nc.tensor.matmul(ps_h1[:], w0_t[:], wT_t[:], start=True, stop=True)
        h1_t = work.tile([64, E], BF16)
        nc.scalar.activation(
            h1_t[:], ps_h1[:], mybir.ActivationFunctionType.Relu,
            bias=b0c_t[:], scale=1.0,
        )

        # persistent PSUM accumulators
        psA = ps_acc.tile([128, E], F32, tag="A")
        psB = ps_acc.tile([64, E], F32, tag="B")
        psC = [ps_acc.tile([64, E], F32, tag=f"C{i}", name=f"psC{i}")
               for i in range(3)]
        psD = ps_acc.tile([128, E], F32, tag="D")

        # bias chunks open each accumulation group
        f3 = fin0_t[:].rearrange("p (b e) -> p b e", b=6)
        nc.tensor.matmul(psA[:], ba_t[:], s1t_t[:], start=True, stop=False,
                         skip_group_check=True)
        nc.tensor.matmul(psB[:], bb_t[:], s1t_t[:], start=True, stop=False,
                         skip_group_check=True)
        for i in range(3):
            nc.tensor.matmul(psC[i][:], bc_t[:], f3[0:64, 2 + i, :], start=True,
                             stop=False, skip_group_check=True)
        nc.tensor.matmul(psD[:], bd_t[:], f3[0:64, 5, :], start=True, stop=False,
                         skip_group_check=True)

        # main loop: 32 chunks, each = 2 k-values; one broadcast serves all
        # six path operands.
        for g in range(G):
            ps_bc = ps_rot.tile([128, E], F32, tag="rot")
            nc.tensor.matmul(ps_bc[:], wg1_t[:, bass.ts(g, 128)], h1_t[:],
                             start=True, stop=True, skip_group_check=True)
            bct = bct_pool.tile([128, E], BF16, tag="bct")
            nc.scalar.activation(
                bct[:], ps_bc[:], mybir.ActivationFunctionType.Relu,
                bias=bg1_t[:, g : g + 1], scale=1.0,
            )
            # fused Khatri-Rao products: pt[:, j*E:(j+1)*E] = fin0_j * bct
            pt = pt_pool.tile([128, 6 * E], BF16, tag="pt")
            bct_b = dataclasses.replace(
                bct[:], ap=[bct[:].ap[0], [0, 6], [1, E]]
            )
            nc.vector.tensor_mul(
                pt[:].rearrange("p (b e) -> p b e", b=6), f3, bct_b
            )
            last = g == G - 1
            p3 = pt[:].rearrange("p (b e) -> p b e", b=6)
            nc.tensor.matmul(psA[:], walo_t[:, bass.ts(g, 128)], p3[:, 0, :],
                             start=False, stop=False, skip_group_check=True)
            nc.tensor.matmul(psA[:], wahi_t[:, bass.ts(g, 128)], p3[:, 1, :],
                             start=False, stop=last, skip_group_check=True)
            nc.tensor.matmul(psB[:], wblo_t[:, bass.ts(g, 64)], p3[:, 0, :],
                             start=False, stop=False, skip_group_check=True)
            nc.tensor.matmul(psB[:], wbhi_t[:, bass.ts(g, 64)], p3[:, 1, :],
                             start=False, stop=last, skip_group_check=True)
            for i in range(3):
                nc.tensor.matmul(psC[i][:], wc_t[:, bass.ts(g, 64)],
                                 p3[:, 2 + i, :], start=False, stop=last,
                                 skip_group_check=True)
            nc.tensor.matmul(psD[:], wd_t[:, bass.ts(g, 128)], p3[:, 5, :],
                             start=False, stop=last, skip_group_check=True)

        # epilogue (still transposed):
        # out0T = (psA*s2) + I3*psD + bias ;  out1T_i = psB*v2_i + psC_i
        tA = work.tile([128, E], F32)
        nc.vector.tensor_mul(tA[:], psA[:], s2b_t[:])
        tD = work.tile([128, E], F32)
        nc.scalar.mul(tD[:], psD[:], I3)
        out0T = work.tile([128, E], F32)
        nc.vector.affine_then_add(out0T[:], tA[:], tD[:], scale=1.0,
                                  bias=bcol_t[:])
        out1T = []
        for i in range(3):
            tB = work.tile([64, E], F32, tag=f"tB{i}")
            nc.vector.tensor_mul(tB[:], psB[:], v2b_t[i][:])
            o1 = work.tile([64, E], F32, tag=f"o1{i}")
            nc.vector.affine_then_add(o1[:], tB[:], psC[i][:], scale=1.0,
                                      bias=0.0)
            out1T.append(o1)

        # transpose back to [E, 320] and store
        for et in range(E // 128):
            sl = bass.ts(et, 128)
            outS = out_pool.tile([128, 320], F32, tag="outS")
            ps_t0 = ps_rot.tile([128, E], F32, tag="rot")
            nc.tensor.transpose(ps_t0[:, 0:128], out0T[:, sl], ident_t[:])
            nc.scalar.copy(outS[:, 0:128], ps_t0[:, 0:128])
            o1v = outS[:, 128:320].rearrange("p (w i) -> p i w", i=3)
            for i in range(3):
                ps_ti = ps_rot.tile([128, E], F32, tag="rot")
                nc.tensor.transpose(ps_ti[:, 0:64], out1T[i][:, sl],
                                    ident_t[0:64, 0:64])
                nc.scalar.copy(o1v[:, i, :], ps_ti[:, 0:64])
            nc.sync.dma_start(outp_d[sl, :], outS[:])

    nc.compile()
    return nc


_NC = None


def _get_nc():
    global _NC
    if _NC is None:
        _NC = _build_nc()
    return _NC


def _prep_inputs(data_in1, data_in2, weight, W0, b0, W1, b1, W2, b2, bias):
    f32 = np.float32
    data_in1 = np.ascontiguousarray(data_in1, dtype=f32)
    data_in2 = np.ascontiguousarray(data_in2, dtype=f32)
    weight = np.ascontiguousarray(weight, dtype=f32)
    W0 = np.asarray(W0, f32); b0 = np.asarray(b0, f32)
    W1 = np.asarray(W1, f32); b1 = np.asarray(b1, f32)
    W2 = np.asarray(W2, f32); b2 = np.asarray(b2, f32)
    bias = np.asarray(bias, f32)

    s1 = data_in1[:, :MUL0]                      # [N,128]
    v1 = data_in1[:, MUL0:].reshape(N, MUL1, 3)  # [N,64,3]
    s2 = data_in2[:, 0]                          # [N]
    v2 = data_in2[:, 1:4]                        # [N,3]

    def bf(x):
        return np.ascontiguousarray(x, dtype=f32).astype(BF16_NP)

    s1t = s1.T                                   # [128,N] f32
    # fused TT operand blocks, each [128, N]
    s1lo = np.concatenate([s1t[0:64], s1t[0:64]], axis=0)
    s1hi = np.concatenate([s1t[64:128], s1t[64:128]], axis=0)
    vs = []
    for i in range(3):
        v1s2 = (v1[:, :, i] * s2[:, None]).T     # [64,N]
        vs.append(np.concatenate([v1s2, v1s2], axis=0))
    dot12 = np.einsum("eui,ei->eu", v1, v2).T    # [64,N]
    d2 = np.concatenate([dot12, dot12], axis=0)
    fin0 = bf(np.stack([s1lo, s1hi, vs[0], vs[1], vs[2], d2], axis=1))
    # fin0: [128, 6, N]
    wT = bf(weight.T)
    s2b = np.ascontiguousarray(np.broadcast_to(s2, (128, N)), dtype=f32)
    v2b = [
        np.ascontiguousarray(np.broadcast_to(v2[:, i], (64, N)), dtype=f32)
        for i in range(3)
    ]

    # W2 chunk layouts: chunk g rows r=(koff*64+uu) = W2x[2g+koff, sel(uu), :]
    def chunks(arr3, usel):  # arr3 [64,U,W] -> [128, G*W]
        a = arr3.reshape(G, 2, arr3.shape[1], arr3.shape[2])[:, :, usel, :]
        return bf(np.transpose(a, (1, 2, 0, 3)).reshape(128, -1))

    Wa3 = W2[:, :N1].reshape(64, 128, 128)
    Wb3 = W2[:, N1:N1 + N2].reshape(64, 128, 64)
    Wc3 = W2[:, N1 + N2:N1 + N2 + N3].reshape(64, 64, 64)
    Wd3 = W2[:, N1 + N2 + N3:].reshape(64, 64, 128)
    lo, hi = slice(0, 64), slice(64, 128)

    shared = {
        "w0": bf(W0),
        "b0c": np.ascontiguousarray(b0.reshape(64, 1), f32),
        "wg1": bf(np.repeat(W1, 64, axis=1)),
        "bg1": np.concatenate(
            [np.broadcast_to(b1[0::2], (64, G)),
             np.broadcast_to(b1[1::2], (64, G))], axis=0).astype(f32),
        "walo": chunks(Wa3, lo),
        "wahi": chunks(Wa3, hi),
        "wblo": chunks(Wb3, lo),
        "wbhi": chunks(Wb3, hi),
        "wc": chunks(Wc3, lo),
        "wd": chunks(Wd3, lo),
        "ba": bf(b2[:N1].reshape(128, 128)),
        "bb": bf(b2[N1:N1 + N2].reshape(128, 64)),
        "bc": bf(b2[N1 + N2:N1 + N2 + N3].reshape(64, 64)),
        "bd": bf(b2[N1 + N2 + N3:].reshape(64, 128)),
        "bcol": np.ascontiguousarray(bias.reshape(128, 1), f32),
        "ident": np.eye(128, dtype=f32),
    }

    in_maps = []
    for c in range(N_CORES):
        e0 = c * E
        m = dict(shared)
        m["s1t"] = bf(s1t[:, e0:e0 + E])
        m["fin0"] = np.ascontiguousarray(
            fin0[:, :, e0:e0 + E]).reshape(128, 6 * E)
        m["wT"] = np.ascontiguousarray(wT[:, e0:e0 + E])
        m["s2b"] = np.ascontiguousarray(s2b[:, e0:e0 + E])
        for i in range(3):
            m[f"v2b{i}"] = np.ascontiguousarray(v2b[i][:, e0:e0 + E])
        in_maps.append(m)
    return in_maps


def run(in_maps, **kwargs):
    nc = _get_nc()
    return run_bass_kernel_spmd(nc, in_maps, list(range(N_CORES)), **kwargs)


def kernel(data_in1, data_in2, weight, W0, b0, W1, b1, W2, b2, bias):
    in_maps = _prep_inputs(
        data_in1, data_in2, weight, W0, b0, W1, b1, W2, b2, bias
    )
    res = run(in_maps)
    out = np.concatenate(
        [np.asarray(res.results[c]["outp"]) for c in range(N_CORES)], axis=0
    )
    return out.astype(np.float32)



# revision 4
# speedup vs baseline: 1.2627x; 1.2627x over previous
"""Trainium2 Bass kernel for nn_O3TensorProductWeighted.

Computes, for each sample e:
    h  = relu(relu(weight @ W0 + b0) @ W1 + b1)           # [64]
    w  = h @ W2 + b2                                      # [36864] (never materialized)
    out0 = PW0*(einsum(Wa,s1)*s2 + I3*einsum(Wd,dot12))
    out1 = PW1*I3*(einsum(Wb,s1) x v2 + einsum(Wc,v1)*s2)
    out  = concat(out0, out1)/SQRT_K ; out[:128] += bias

Strategy: reassociate each einsum against the (k,u)-joint contraction of the
per-sample Khatri-Rao product h (x) x, with the joint index on SBUF
partitions and SAMPLES on the PSUM partition dim ("transposed" layout):
    psum[sample, w] += pt[(k,u), sample].T @ W2chunk[(k,u), w]
Every path matmul then runs at full 128x128 PE utilization (cost scales
with the *output feature width*, so the 64-wide B/C paths cost half), and
the output lands directly in [sample, 320] layout - no final transposes.
Per chunk, one PE broadcast matmul replicates 2 h-values across 64
partitions, Act evacuates it through the relu, and the six per-sample
product blocks are split DVE(4)/Pool(2). b2-bias terms are static prologue
matmuls into the same psum accumulators; s2/v2 scaling happens in a 4-op
DVE epilogue per 128-sample block via per-partition scale APs.
512 samples per core, pure data parallel over 8 cores.
"""

import dataclasses
import sys

sys.path.insert(0, "/opt/trn_rl_repo")

from contextlib import ExitStack

import ml_dtypes
import numpy as np

import concourse.bacc as bacc
import concourse.bass as bass
import concourse.tile as tile
from concourse import mybir
from concourse.bass_utils import run_bass_kernel_spmd

BF16 = mybir.dt.bfloat16
F32 = mybir.dt.float32
BF16_NP = ml_dtypes.bfloat16

N_CORES = 8
N = 4096
E = N // N_CORES  # 512 samples per core
NB = E // 128     # 4 sample blocks per core

MUL0, MUL1 = 128, 64
N1 = MUL0 * MUL0          # 16384
N2 = MUL0 * MUL1          # 8192
N3 = MUL1 * MUL1          # 4096
FAN = MUL0 + MUL1         # 192
I3 = float(1.0 / np.sqrt(3.0))
# PW0/SQRT_K == 1.0 and PW1*I3/SQRT_K == 1.0 exactly; only I3 remains on D
# (folded into wd/bd on the host).

G = 32  # chunks; chunk g covers k in {2g, 2g+1} x 64 u-values (128 rows)


def _build_nc():
    nc = bacc.Bacc(None)

    # per-core inputs
    # fused product operand: [s1lo2 | s1hi2 | vs0 | vs1 | vs2 | d2], each [128, E]
    fin0_d = nc.declare_dram_parameter("fin0", [128, 6 * E], BF16, isOutput=False)
    s1t_d = nc.declare_dram_parameter("s1t", [128, E], BF16, isOutput=False)
    wT_d = nc.declare_dram_parameter("wT", [16, E], BF16, isOutput=False)
    s2c_d = nc.declare_dram_parameter("s2c", [128, NB], F32, isOutput=False)
    v2c_d = nc.declare_dram_parameter("v2c", [128, 3 * NB], F32, isOutput=False)

    # replicated parameters
    w0_d = nc.declare_dram_parameter("w0", [16, 64], BF16, isOutput=False)
    b0c_d = nc.declare_dram_parameter("b0c", [64, 1], F32, isOutput=False)
    wg1_d = nc.declare_dram_parameter("wg1", [64, G * 128], BF16, isOutput=False)
    bg1_d = nc.declare_dram_parameter("bg1", [128, G], F32, isOutput=False)
    # W2 chunk tensors, rhs layout [joint(koff,u) rows, w cols]
    wablo_d = nc.declare_dram_parameter("wablo", [128, G * 192], BF16, isOutput=False)
    wabhi_d = nc.declare_dram_parameter("wabhi", [128, G * 192], BF16, isOutput=False)
    wc_d = nc.declare_dram_parameter("wc", [128, G * 64], BF16, isOutput=False)
    wd_d = nc.declare_dram_parameter("wd", [128, G * 128], BF16, isOutput=False)
    # b2 bias blocks (rhs of prologue matmuls)
    ba_d = nc.declare_dram_parameter("ba", [128, 128], BF16, isOutput=False)
    bb_d = nc.declare_dram_parameter("bb", [128, 64], BF16, isOutput=False)
    bc_d = nc.declare_dram_parameter("bc", [64, 64], BF16, isOutput=False)
    bd_d = nc.declare_dram_parameter("bd", [64, 128], BF16, isOutput=False)
    # final bias via rank-1 matmul: ones[1,128-samples] x biasrow[1,128-w]
    ones_d = nc.declare_dram_parameter("ones1", [1, 128], BF16, isOutput=False)
    brow_d = nc.declare_dram_parameter("brow", [1, 128], BF16, isOutput=False)

    outp_d = nc.declare_dram_parameter("outp", [E, 320], F32, isOutput=True)

    with tile.TileContext(nc) as tc, ExitStack() as ctx:
        const = ctx.enter_context(tc.tile_pool(name="const", bufs=1))
        work = ctx.enter_context(tc.tile_pool(name="work", bufs=1))
        bct_pool = ctx.enter_context(tc.tile_pool(name="bct", bufs=4))
        pt_pool = ctx.enter_context(tc.tile_pool(name="pt", bufs=3))
        out_pool = ctx.enter_context(tc.tile_pool(name="outs", bufs=2))
        ps_acc = ctx.enter_context(tc.tile_pool(name="ps_acc", bufs=1, space="PSUM"))
        ps_rot = ctx.enter_context(tc.tile_pool(name="ps_rot", bufs=3, space="PSUM"))

        def load(dparam):
            t = const.tile(dparam.shape, dparam.dtype, name=f"t_{dparam.name}")
            nc.sync.dma_start(t[:], dparam[:])
            return t

        # small inputs first so the MLP + prologue can start immediately
        wT_t = load(wT_d)
        w0_t = load(w0_d)
        b0c_t = load(b0c_d)
        s1t_t = load(s1t_d)
        ba_t = load(ba_d)
        bb_t = load(bb_d)
        bc_t = load(bc_d)
        bd_t = load(bd_d)
        ones_t = load(ones_d)
        brow_t = load(brow_d)
        wg1_t = load(wg1_d)
        bg1_t = load(bg1_d)
        fin0_t = load(fin0_d)
        s2c_t = load(s2c_d)
        v2c_t = load(v2c_d)
        # W2 chunk tensors: quarter the DMAs, interleaved in consumption
        # order, so the first chunks' weights arrive before the PE needs them
        wablo_t = const.tile([128, G * 192], BF16, name="t_wablo")
        wabhi_t = const.tile([128, G * 192], BF16, name="t_wabhi")
        wc_t = const.tile([128, G * 64], BF16, name="t_wc")
        wd_t = const.tile([128, G * 128], BF16, name="t_wd")
        for q in range(4):
            sl192 = bass.ts(q, G * 48)
            sl64 = bass.ts(q, G * 16)
            sl128 = bass.ts(q, G * 32)
            nc.sync.dma_start(wablo_t[:, sl192], wablo_d[:, sl192])
            nc.sync.dma_start(wabhi_t[:, sl192], wabhi_d[:, sl192])
            nc.sync.dma_start(wc_t[:, sl64], wc_d[:, sl64])
            nc.sync.dma_start(wd_t[:, sl128], wd_d[:, sl128])

        # MLP layer 1: h1 = relu(W0.T @ wT + b0) : [64, E]
        ps_h1 = ps_rot.tile([64, E], F32, tag="rot", name="ps_h1")
        nc.tensor.matmul(ps_h1[:], w0_t[:], wT_t[:], start=True, stop=True,
                         skip_group_check=True)
        h1_t = work.tile([64, E], BF16)
        nc.scalar.activation(
            h1_t[:], ps_h1[:], mybir.ActivationFunctionType.Relu,
            bias=b0c_t[:], scale=1.0,
        )

        # persistent PSUM accumulators: one bank per 128-sample block,
        # free layout: A 0:128 | B 128:192 | C0..2 192:384 | D 384:512
        psblk = [ps_acc.tile([128, 512], F32, tag=f"blk{b}", name=f"psblk{b}")
                 for b in range(NB)]

        # prologue: b2-bias terms open each bank (first matmul start=True
        # marks the whole 2KB zero-region; everything later accumulates)
        f3 = fin0_t[:].rearrange("p (b e) -> p b e", b=6)
        for b in range(NB):
            blk = bass.ts(b, 128)
            nc.tensor.matmul(psblk[b][:, 0:128], s1t_t[:, blk], ba_t[:],
                             start=True, stop=False, skip_group_check=True)
            nc.tensor.matmul(psblk[b][:, 128:192], s1t_t[:, blk], bb_t[:],
                             start=False, stop=False, skip_group_check=True)
            for i in range(3):
                nc.tensor.matmul(psblk[b][:, 192 + 64 * i:256 + 64 * i],
                                 f3[0:64, 2 + i, blk], bc_t[:],
                                 start=False, stop=False, skip_group_check=True)
            nc.tensor.matmul(psblk[b][:, 384:512], f3[0:64, 5, blk], bd_t[:],
                             start=False, stop=False, skip_group_check=True)
            nc.tensor.matmul(psblk[b][:, 384:512], ones_t[:], brow_t[:],
                             start=False, stop=False, skip_group_check=True)

        # main loop: 32 chunks, each = 2 k-values broadcast to 128 rows
        for g in range(G):
            last = g == G - 1
            ps_bc = ps_rot.tile([128, E], F32, tag="rot")
            nc.tensor.matmul(ps_bc[:], wg1_t[:, bass.ts(g, 128)], h1_t[:],
                             start=True, stop=True, skip_group_check=True)
            bct = bct_pool.tile([128, E], BF16, tag="bct")
            nc.scalar.activation(
                bct[:], ps_bc[:], mybir.ActivationFunctionType.Relu,
                bias=bg1_t[:, g:g + 1], scale=1.0,
            )
            # per-sample products pt[:, j*E:(j+1)*E] = fin0_j * bct
            # blocks 0-3 on DVE, blocks 4-5 on Pool
            pt = pt_pool.tile([128, 6 * E], BF16, tag="pt")
            p6 = pt[:].rearrange("p (b e) -> p b e", b=6)
            bct4 = dataclasses.replace(
                bct[:], ap=[bct[:].ap[0], [0, 4], [1, E]]
            )
            bct2 = dataclasses.replace(
                bct[:], ap=[bct[:].ap[0], [0, 2], [1, E]]
            )
            nc.vector.tensor_mul(
                pt[:, 0:4 * E].rearrange("p (b e) -> p b e", b=4),
                fin0_t[:, 0:4 * E].rearrange("p (b e) -> p b e", b=4), bct4)
            nc.gpsimd.tensor_mul(
                pt[:, 4 * E:6 * E].rearrange("p (b e) -> p b e", b=2),
                fin0_t[:, 4 * E:6 * E].rearrange("p (b e) -> p b e", b=2), bct2)

            for b in range(NB):
                blk = bass.ts(b, 128)
                nc.tensor.matmul(psblk[b][:, 0:192], p6[:, 0, blk],
                                 wablo_t[:, bass.ts(g, 192)],
                                 start=False, stop=False, skip_group_check=True)
                nc.tensor.matmul(psblk[b][:, 0:192], p6[:, 1, blk],
                                 wabhi_t[:, bass.ts(g, 192)],
                                 start=False, stop=last, skip_group_check=True)
                for i in range(3):
                    nc.tensor.matmul(psblk[b][:, 192 + 64 * i:256 + 64 * i],
                                     p6[:, 2 + i, blk], wc_t[:, bass.ts(g, 64)],
                                     start=False, stop=last,
                                     skip_group_check=True)
                nc.tensor.matmul(psblk[b][:, 384:512], p6[:, 5, blk],
                                 wd_t[:, bass.ts(g, 128)],
                                 start=False, stop=last, skip_group_check=True)

        # epilogue per block: out0 = psA*s2 + psD ; out1_i = psB*v2_i + psC_i
        # output is already in [sample, 320] layout - no transposes
        for b in range(NB):
            outS = out_pool.tile([128, 320], F32, tag="outS")
            nc.vector.affine_then_add(outS[:, 0:128], psblk[b][:, 0:128],
                                      psblk[b][:, 384:512],
                                      scale=s2c_t[:, b:b + 1], bias=0.0)
            o1v = outS[:, 128:320].rearrange("p (w i) -> p i w", i=3)
            for i in range(3):
                nc.vector.affine_then_add(
                    o1v[:, i, :], psblk[b][:, 128:192],
                    psblk[b][:, 192 + 64 * i:256 + 64 * i],
                    scale=v2c_t[:, 3 * b + i:3 * b + i + 1], bias=0.0)
            nc.sync.dma_start(outp_d[bass.ts(b, 128), :], outS[:])

    nc.compile()
    return nc


_NC = None


def _get_nc():
    global _NC
    if _NC is None:
        _NC = _build_nc()
    return _NC


def _prep_inputs(data_in1, data_in2, weight, W0, b0, W1, b1, W2, b2, bias):
    f32 = np.float32
    data_in1 = np.ascontiguousarray(data_in1, dtype=f32)
    data_in2 = np.ascontiguousarray(data_in2, dtype=f32)
    weight = np.ascontiguousarray(weight, dtype=f32)
    W0 = np.asarray(W0, f32); b0 = np.asarray(b0, f32)
    W1 = np.asarray(W1, f32); b1 = np.asarray(b1, f32)
    W2 = np.asarray(W2, f32); b2 = np.asarray(b2, f32)
    bias = np.asarray(bias, f32)

    s1 = data_in1[:, :MUL0]                      # [N,128]
    v1 = data_in1[:, MUL0:].reshape(N, MUL1, 3)  # [N,64,3]
    s2 = data_in2[:, 0]                          # [N]
    v2 = data_in2[:, 1:4]                        # [N,3]

    def bf(x):
        return np.ascontiguousarray(x, dtype=f32).astype(BF16_NP)

    s1t = s1.T                                   # [128,N] f32
    # fused product operand blocks, each [128, N]
    s1lo = np.concatenate([s1t[0:64], s1t[0:64]], axis=0)
    s1hi = np.concatenate([s1t[64:128], s1t[64:128]], axis=0)
    vs = []
    for i in range(3):
        v1s2 = (v1[:, :, i] * s2[:, None]).T     # [64,N]
        vs.append(np.concatenate([v1s2, v1s2], axis=0))
    dot12 = np.einsum("eui,ei->eu", v1, v2).T    # [64,N]
    d2 = np.concatenate([dot12, dot12], axis=0)
    fin0 = bf(np.stack([s1lo, s1hi, vs[0], vs[1], vs[2], d2], axis=1))
    # fin0: [128, 6, N]
    wT = bf(weight.T)

    # W2 chunk layouts: chunk g rows r=(koff*64+uu) = W2x[2g+koff, sel(uu), :]
    def chunks(arr3, usel):  # arr3 [64,U,W] -> [128, G, W]
        a = arr3.reshape(G, 2, arr3.shape[1], arr3.shape[2])[:, :, usel, :]
        return np.transpose(a, (1, 2, 0, 3)).reshape(128, G, arr3.shape[2])

    Wa3 = W2[:, :N1].reshape(64, 128, 128)
    Wb3 = W2[:, N1:N1 + N2].reshape(64, 128, 64)
    Wc3 = W2[:, N1 + N2:N1 + N2 + N3].reshape(64, 64, 64)
    Wd3 = W2[:, N1 + N2 + N3:].reshape(64, 64, 128) * I3
    lo, hi = slice(0, 64), slice(64, 128)

    wablo = bf(np.concatenate(
        [chunks(Wa3, lo), chunks(Wb3, lo)], axis=2).reshape(128, G * 192))
    wabhi = bf(np.concatenate(
        [chunks(Wa3, hi), chunks(Wb3, hi)], axis=2).reshape(128, G * 192))
    wc = bf(chunks(Wc3, lo).reshape(128, G * 64))
    wd = bf(chunks(Wd3, lo).reshape(128, G * 128))

    shared = {
        "w0": bf(W0),
        "b0c": np.ascontiguousarray(b0.reshape(64, 1), f32),
        "wg1": bf(np.repeat(W1, 64, axis=1)),
        "bg1": np.concatenate(
            [np.broadcast_to(b1[0::2], (64, G)),
             np.broadcast_to(b1[1::2], (64, G))], axis=0).astype(f32),
        "wablo": wablo,
        "wabhi": wabhi,
        "wc": wc,
        "wd": wd,
        "ba": bf(b2[:N1].reshape(128, 128)),
        "bb": bf(b2[N1:N1 + N2].reshape(128, 64)),
        "bc": bf(b2[N1 + N2:N1 + N2 + N3].reshape(64, 64)),
        "bd": bf(b2[N1 + N2 + N3:].reshape(64, 128) * I3),
        "ones1": bf(np.ones((1, 128))),
        "brow": bf(bias.reshape(1, 128)),
    }

    in_maps = []
    for c in range(N_CORES):
        e0 = c * E
        m = dict(shared)
        m["fin0"] = np.ascontiguousarray(
            fin0[:, :, e0:e0 + E]).reshape(128, 6 * E)
        m["s1t"] = bf(s1t[:, e0:e0 + E])
        m["wT"] = np.ascontiguousarray(wT[:, e0:e0 + E])
        m["s2c"] = np.ascontiguousarray(
            s2[e0:e0 + E].reshape(NB, 128).T, f32)
        m["v2c"] = np.ascontiguousarray(
            v2[e0:e0 + E].reshape(NB, 128, 3).transpose(1, 0, 2).reshape(
                128, 3 * NB), f32)
        in_maps.append(m)
    return in_maps


def run(in_maps, **kwargs):
    nc = _get_nc()
    return run_bass_kernel_spmd(nc, in_maps, list(range(N_CORES)), **kwargs)


def kernel(data_in1, data_in2, weight, W0, b0, W1, b1, W2, b2, bias):
    in_maps = _prep_inputs(
        data_in1, data_in2, weight, W0, b0, W1, b1, W2, b2, bias
    )
    res = run(in_maps)
    out = np.concatenate(
        [np.asarray(res.results[c]["outp"]) for c in range(N_CORES)], axis=0
    )
    return out.astype(np.float32)


# revision 6
# speedup vs baseline: 1.4465x; 1.1455x over previous
"""Trainium2 Bass kernel for nn_O3TensorProductWeighted.

Computes, for each sample e:
    h  = relu(relu(weight @ W0 + b0) @ W1 + b1)           # [64]
    w  = h @ W2 + b2                                      # [36864] (never materialized)
    out0 = PW0*(einsum(Wa,s1)*s2 + I3*einsum(Wd,dot12))
    out1 = PW1*I3*(einsum(Wb,s1) x v2 + einsum(Wc,v1)*s2)
    out  = concat(out0, out1)/SQRT_K ; out[:128] += bias

Strategy: reassociate each einsum against the (k,u)-joint contraction of the
per-sample Khatri-Rao product h (x) x, with the joint index on SBUF
partitions and SAMPLES on the PSUM partition dim ("transposed" layout):
    psum[sample, w] += pt[(k,u), sample].T @ W2chunk[(k,u), w]
Every path matmul then runs at full 128x128 PE utilization (cost scales
with the *output feature width*, so the 64-wide B/C paths cost half), and
the output lands directly in [sample, 320] layout - no final transposes.
Per chunk, one PE broadcast matmul replicates 2 h-values across 64
partitions, Act evacuates it through the relu, and the six per-sample
product blocks are split DVE(4)/Pool(2). b2-bias terms are static prologue
matmuls into the same psum accumulators; s2/v2 scaling happens in a 4-op
DVE epilogue per 128-sample block via per-partition scale APs.
512 samples per core, pure data parallel over 8 cores.
"""

import dataclasses
import sys

sys.path.insert(0, "/opt/trn_rl_repo")

from contextlib import ExitStack

import ml_dtypes
import numpy as np

import concourse.bacc as bacc
import concourse.bass as bass
import concourse.tile as tile
from concourse import mybir
from concourse.bass_utils import run_bass_kernel_spmd

BF16 = mybir.dt.bfloat16
F32 = mybir.dt.float32
BF16_NP = ml_dtypes.bfloat16

N_CORES = 8
N = 4096
E = N // N_CORES  # 512 samples per core
NB = E // 128     # 4 sample blocks per core

MUL0, MUL1 = 128, 64
N1 = MUL0 * MUL0          # 16384
N2 = MUL0 * MUL1          # 8192
N3 = MUL1 * MUL1          # 4096
FAN = MUL0 + MUL1         # 192
I3 = float(1.0 / np.sqrt(3.0))
# PW0/SQRT_K == 1.0 and PW1*I3/SQRT_K == 1.0 exactly; only I3 remains on D
# (folded into wd/bd on the host).

G = 32  # chunks; chunk g covers k in {2g, 2g+1} x 64 u-values (128 rows)


def _build_nc():
    nc = bacc.Bacc(None)

    # per-core inputs
    # fused product operand: [s1lo2 | s1hi2 | vs0 | vs1 | vs2 | d2], each [128, E]
    fin0_d = nc.declare_dram_parameter("fin0", [128, 6 * E], BF16, isOutput=False)
    s1t_d = nc.declare_dram_parameter("s1t", [128, E], BF16, isOutput=False)
    wT_d = nc.declare_dram_parameter("wT", [16, E], BF16, isOutput=False)
    s2c_d = nc.declare_dram_parameter("s2c", [128, NB], F32, isOutput=False)
    v2c_d = nc.declare_dram_parameter("v2c", [128, 3 * NB], F32, isOutput=False)

    # replicated parameters
    w0_d = nc.declare_dram_parameter("w0", [16, 64], BF16, isOutput=False)
    b0c_d = nc.declare_dram_parameter("b0c", [64, 1], F32, isOutput=False)
    wg1_d = nc.declare_dram_parameter("wg1", [64, G * 128], BF16, isOutput=False)
    bg1_d = nc.declare_dram_parameter("bg1", [128, G], F32, isOutput=False)
    # W2 chunk tensors, rhs layout [joint(koff,u) rows, w cols]
    wablo_d = nc.declare_dram_parameter("wablo", [128, G * 192], BF16, isOutput=False)
    wabhi_d = nc.declare_dram_parameter("wabhi", [128, G * 192], BF16, isOutput=False)
    wc_d = nc.declare_dram_parameter("wc", [128, G * 64], BF16, isOutput=False)
    wd_d = nc.declare_dram_parameter("wd", [128, G * 128], BF16, isOutput=False)
    # b2 bias blocks (rhs of prologue matmuls)
    ba_d = nc.declare_dram_parameter("ba", [128, 128], BF16, isOutput=False)
    bb_d = nc.declare_dram_parameter("bb", [128, 64], BF16, isOutput=False)
    bc_d = nc.declare_dram_parameter("bc", [64, 64], BF16, isOutput=False)
    bd_d = nc.declare_dram_parameter("bd", [64, 128], BF16, isOutput=False)
    # final bias via rank-1 matmul: ones[1,128-samples] x biasrow[1,128-w]
    ones_d = nc.declare_dram_parameter("ones1", [1, 128], BF16, isOutput=False)
    brow_d = nc.declare_dram_parameter("brow", [1, 128], BF16, isOutput=False)

    outp_d = nc.declare_dram_parameter("outp", [E, 320], F32, isOutput=True)

    with tile.TileContext(nc) as tc, ExitStack() as ctx:
        const = ctx.enter_context(tc.tile_pool(name="const", bufs=1))
        work = ctx.enter_context(tc.tile_pool(name="work", bufs=1))
        bct_pool = ctx.enter_context(tc.tile_pool(name="bct", bufs=4))
        pt_pool = ctx.enter_context(tc.tile_pool(name="pt", bufs=3))
        out_pool = ctx.enter_context(tc.tile_pool(name="outs", bufs=2))
        ps_acc = ctx.enter_context(tc.tile_pool(name="ps_acc", bufs=1, space="PSUM"))
        ps_rot = ctx.enter_context(tc.tile_pool(name="ps_rot", bufs=3, space="PSUM"))

        def load(dparam, engine=None):
            t = const.tile(dparam.shape, dparam.dtype, name=f"t_{dparam.name}")
            (engine or nc.sync).dma_start(t[:], dparam[:])
            return t

        # critical-path inputs first: MLP weights, then broadcast weights
        # (chunk 0 slice), then fin0 (products), then first W2 slices
        wT_t = load(wT_d)
        w0_t = load(w0_d)
        b0c_t = load(b0c_d)
        bg1_t = load(bg1_d)
        wg1_t = const.tile([64, G * 128], BF16, name="t_wg1")
        nc.sync.dma_start(wg1_t[:, 0:8 * 128], wg1_d[:, 0:8 * 128])
        fin0_t = load(fin0_d)
        # W2 chunk tensors, sliced in consumption order: chunk groups
        # [0:4][4:8][8:16][16:24][24:32] so early chunks aren't DMA-gated
        wablo_t = const.tile([128, G * 192], BF16, name="t_wablo")
        wabhi_t = const.tile([128, G * 192], BF16, name="t_wabhi")
        wc_t = const.tile([128, G * 64], BF16, name="t_wc")
        wd_t = const.tile([128, G * 128], BF16, name="t_wd")
        gslices = [(0, 4), (4, 8), (8, 16), (16, 24), (24, 32)]
        for si, (g0, g1) in enumerate(gslices):
            nc.sync.dma_start(wablo_t[:, g0 * 192:g1 * 192],
                              wablo_d[:, g0 * 192:g1 * 192])
            nc.sync.dma_start(wabhi_t[:, g0 * 192:g1 * 192],
                              wabhi_d[:, g0 * 192:g1 * 192])
            nc.sync.dma_start(wc_t[:, g0 * 64:g1 * 64],
                              wc_d[:, g0 * 64:g1 * 64])
            nc.sync.dma_start(wd_t[:, g0 * 128:g1 * 128],
                              wd_d[:, g0 * 128:g1 * 128])
            if si == 0:
                # rest of the broadcast weights + non-urgent small inputs
                nc.sync.dma_start(wg1_t[:, 8 * 128:], wg1_d[:, 8 * 128:])
                s1t_t = load(s1t_d)
                ba_t = load(ba_d)
                bb_t = load(bb_d)
                bc_t = load(bc_d)
                bd_t = load(bd_d)
                ones_t = load(ones_d)
                brow_t = load(brow_d)
                s2c_t = load(s2c_d)
                v2c_t = load(v2c_d)

        # MLP layer 1: h1 = relu(W0.T @ wT + b0) : [64, E]
        ps_h1 = ps_rot.tile([64, E], F32, tag="rot", name="ps_h1")
        nc.tensor.matmul(ps_h1[:], w0_t[:], wT_t[:], start=True, stop=True,
                         skip_group_check=True)
        h1_t = work.tile([64, E], BF16)
        nc.scalar.activation(
            h1_t[:], ps_h1[:], mybir.ActivationFunctionType.Relu,
            bias=b0c_t[:], scale=1.0,
        )

        # persistent PSUM accumulators: one bank per 128-sample block,
        # free layout: A 0:128 | B 128:192 | C0..2 192:384 | D 384:512
        psblk = [ps_acc.tile([128, 512], F32, tag=f"blk{b}", name=f"psblk{b}")
                 for b in range(NB)]

        f3 = fin0_t[:].rearrange("p (b e) -> p b e", b=6)

        def bcast_stage(g):
            """broadcast 2 h-values to 128 rows, relu-evac, form products"""
            ps_bc = ps_rot.tile([128, E], F32, tag="rot")
            nc.tensor.matmul(ps_bc[:], wg1_t[:, bass.ts(g, 128)], h1_t[:],
                             start=True, stop=True, skip_group_check=True)
            bct = bct_pool.tile([128, E], BF16, tag="bct")
            nc.scalar.activation(
                bct[:], ps_bc[:], mybir.ActivationFunctionType.Relu,
                bias=bg1_t[:, g:g + 1], scale=1.0,
            )
            # per-sample products pt[:, j*E:(j+1)*E] = fin0_j * bct
            # blocks 0-3 on DVE, blocks 4-5 on Pool
            pt = pt_pool.tile([128, 6 * E], BF16, tag="pt")
            bct4 = dataclasses.replace(
                bct[:], ap=[bct[:].ap[0], [0, 4], [1, E]]
            )
            bct2 = dataclasses.replace(
                bct[:], ap=[bct[:].ap[0], [0, 2], [1, E]]
            )
            nc.vector.tensor_mul(
                pt[:, 0:4 * E].rearrange("p (b e) -> p b e", b=4),
                fin0_t[:, 0:4 * E].rearrange("p (b e) -> p b e", b=4), bct4)
            nc.gpsimd.tensor_mul(
                pt[:, 4 * E:6 * E].rearrange("p (b e) -> p b e", b=2),
                fin0_t[:, 4 * E:6 * E].rearrange("p (b e) -> p b e", b=2),
                bct2)
            return pt

        def path_matmuls(g, pt):
            """joint-contraction matmuls: psum[sample, w] += pt.T @ W2chunk.
            chunk 0 opens each psum bank (start=True marks the whole
            2KB zero-region; all later matmuls accumulate)."""
            first = g == 0
            p6 = pt[:].rearrange("p (b e) -> p b e", b=6)
            for b in range(NB):
                blk = bass.ts(b, 128)
                nc.tensor.matmul(psblk[b][:, 0:192], p6[:, 0, blk],
                                 wablo_t[:, bass.ts(g, 192)],
                                 start=first, stop=False,
                                 skip_group_check=True)
                nc.tensor.matmul(psblk[b][:, 0:192], p6[:, 1, blk],
                                 wabhi_t[:, bass.ts(g, 192)],
                                 start=False, stop=False,
                                 skip_group_check=True)
                for i in range(3):
                    nc.tensor.matmul(psblk[b][:, 192 + 64 * i:256 + 64 * i],
                                     p6[:, 2 + i, blk],
                                     wc_t[:, bass.ts(g, 64)],
                                     start=False, stop=False,
                                     skip_group_check=True)
                nc.tensor.matmul(psblk[b][:, 384:512], p6[:, 5, blk],
                                 wd_t[:, bass.ts(g, 128)],
                                 start=False, stop=False,
                                 skip_group_check=True)

        # software-pipelined main loop: broadcast/products run 2 chunks
        # ahead of the path matmuls so the PE never waits on the
        # relu->products chain
        pts = {0: bcast_stage(0), 1: bcast_stage(1)}
        for g in range(G):
            if g + 2 < G:
                pts[g + 2] = bcast_stage(g + 2)
            path_matmuls(g, pts.pop(g))

        # b2-bias terms: plain accumulation, off the startup critical path
        for b in range(NB):
            blk = bass.ts(b, 128)
            last_b = b == NB - 1
            nc.tensor.matmul(psblk[b][:, 0:128], s1t_t[:, blk], ba_t[:],
                             start=False, stop=True, skip_group_check=True)
            nc.tensor.matmul(psblk[b][:, 128:192], s1t_t[:, blk], bb_t[:],
                             start=False, stop=True, skip_group_check=True)
            for i in range(3):
                nc.tensor.matmul(psblk[b][:, 192 + 64 * i:256 + 64 * i],
                                 f3[0:64, 2 + i, blk], bc_t[:],
                                 start=False, stop=True,
                                 skip_group_check=True)
            nc.tensor.matmul(psblk[b][:, 384:512], f3[0:64, 5, blk], bd_t[:],
                             start=False, stop=False, skip_group_check=True)
            nc.tensor.matmul(psblk[b][:, 384:512], ones_t[:], brow_t[:],
                             start=False, stop=True, skip_group_check=True)

        # epilogue per block: out0 = psA*s2 + psD ; out1_i = psB*v2_i + psC_i
        # output is already in [sample, 320] layout - no transposes
        for b in range(NB):
            outS = out_pool.tile([128, 320], F32, tag="outS")
            nc.vector.affine_then_add(outS[:, 0:128], psblk[b][:, 0:128],
                                      psblk[b][:, 384:512],
                                      scale=s2c_t[:, b:b + 1], bias=0.0)
            o1v = outS[:, 128:320].rearrange("p (w i) -> p i w", i=3)
            for i in range(3):
                nc.vector.affine_then_add(
                    o1v[:, i, :], psblk[b][:, 128:192],
                    psblk[b][:, 192 + 64 * i:256 + 64 * i],
                    scale=v2c_t[:, 3 * b + i:3 * b + i + 1], bias=0.0)
            nc.sync.dma_start(outp_d[bass.ts(b, 128), :], outS[:])

    nc.compile()
    return nc


_NC = None


def _get_nc():
    global _NC
    if _NC is None:
        _NC = _build_nc()
    return _NC


def _prep_inputs(data_in1, data_in2, weight, W0, b0, W1, b1, W2, b2, bias):
    f32 = np.float32
    data_in1 = np.ascontiguousarray(data_in1, dtype=f32)
    data_in2 = np.ascontiguousarray(data_in2, dtype=f32)
    weight = np.ascontiguousarray(weight, dtype=f32)
    W0 = np.asarray(W0, f32); b0 = np.asarray(b0, f32)
    W1 = np.asarray(W1, f32); b1 = np.asarray(b1, f32)
    W2 = np.asarray(W2, f32); b2 = np.asarray(b2, f32)
    bias = np.asarray(bias, f32)

    s1 = data_in1[:, :MUL0]                      # [N,128]
    v1 = data_in1[:, MUL0:].reshape(N, MUL1, 3)  # [N,64,3]
    s2 = data_in2[:, 0]                          # [N]
    v2 = data_in2[:, 1:4]                        # [N,3]

    def bf(x):
        return np.ascontiguousarray(x, dtype=f32).astype(BF16_NP)

    s1t = s1.T                                   # [128,N] f32
    # fused product operand blocks, each [128, N]
    s1lo = np.concatenate([s1t[0:64], s1t[0:64]], axis=0)
    s1hi = np.concatenate([s1t[64:128], s1t[64:128]], axis=0)
    vs = []
    for i in range(3):
        v1s2 = (v1[:, :, i] * s2[:, None]).T     # [64,N]
        vs.append(np.concatenate([v1s2, v1s2], axis=0))
    dot12 = np.einsum("eui,ei->eu", v1, v2).T    # [64,N]
    d2 = np.concatenate([dot12, dot12], axis=0)
    fin0 = bf(np.stack([s1lo, s1hi, vs[0], vs[1], vs[2], d2], axis=1))
    # fin0: [128, 6, N]
    wT = bf(weight.T)

    # W2 chunk layouts: chunk g rows r=(koff*64+uu) = W2x[2g+koff, sel(uu), :]
    def chunks(arr3, usel):  # arr3 [64,U,W] -> [128, G, W]
        a = arr3.reshape(G, 2, arr3.shape[1], arr3.shape[2])[:, :, usel, :]
        return np.transpose(a, (1, 2, 0, 3)).reshape(128, G, arr3.shape[2])

    Wa3 = W2[:, :N1].reshape(64, 128, 128)
    Wb3 = W2[:, N1:N1 + N2].reshape(64, 128, 64)
    Wc3 = W2[:, N1 + N2:N1 + N2 + N3].reshape(64, 64, 64)
    Wd3 = W2[:, N1 + N2 + N3:].reshape(64, 64, 128) * I3
    lo, hi = slice(0, 64), slice(64, 128)

    wablo = bf(np.concatenate(
        [chunks(Wa3, lo), chunks(Wb3, lo)], axis=2).reshape(128, G * 192))
    wabhi = bf(np.concatenate(
        [chunks(Wa3, hi), chunks(Wb3, hi)], axis=2).reshape(128, G * 192))
    wc = bf(chunks(Wc3, lo).reshape(128, G * 64))
    wd = bf(chunks(Wd3, lo).reshape(128, G * 128))

    shared = {
        "w0": bf(W0),
        "b0c": np.ascontiguousarray(b0.reshape(64, 1), f32),
        "wg1": bf(np.repeat(W1, 64, axis=1)),
        "bg1": np.concatenate(
            [np.broadcast_to(b1[0::2], (64, G)),
             np.broadcast_to(b1[1::2], (64, G))], axis=0).astype(f32),
        "wablo": wablo,
        "wabhi": wabhi,
        "wc": wc,
        "wd": wd,
        "ba": bf(b2[:N1].reshape(128, 128)),
        "bb": bf(b2[N1:N1 + N2].reshape(128, 64)),
        "bc": bf(b2[N1 + N2:N1 + N2 + N3].reshape(64, 64)),
        "bd": bf(b2[N1 + N2 + N3:].reshape(64, 128) * I3),
        "ones1": bf(np.ones((1, 128))),
        "brow": bf(bias.reshape(1, 128)),
    }

    in_maps = []
    for c in range(N_CORES):
        e0 = c * E
        m = dict(shared)
        m["fin0"] = np.ascontiguousarray(
            fin0[:, :, e0:e0 + E]).reshape(128, 6 * E)
        m["s1t"] = bf(s1t[:, e0:e0 + E])
        m["wT"] = np.ascontiguousarray(wT[:, e0:e0 + E])
        m["s2c"] = np.ascontiguousarray(
            s2[e0:e0 + E].reshape(NB, 128).T, f32)
        m["v2c"] = np.ascontiguousarray(
            v2[e0:e0 + E].reshape(NB, 128, 3).transpose(1, 0, 2).reshape(
                128, 3 * NB), f32)
        in_maps.append(m)
    return in_maps


def run(in_maps, **kwargs):
    nc = _get_nc()
    return run_bass_kernel_spmd(nc, in_maps, list(range(N_CORES)), **kwargs)


def kernel(data_in1, data_in2, weight, W0, b0, W1, b1, W2, b2, bias):
    in_maps = _prep_inputs(
        data_in1, data_in2, weight, W0, b0, W1, b1, W2, b2, bias
    )
    res = run(in_maps)
    out = np.concatenate(
        [np.asarray(res.results[c]["outp"]) for c in range(N_CORES)], axis=0
    )
    return out.astype(np.float32)
